# revision 22
# baseline (speedup 1.0000x reference)
"""Single-head unscaled attention (B=8, T=2048, D=1024, NODES=1024) on 8 trn2 cores.

Sharding: data-parallel over batch — core b computes batch element b end-to-end.
Weights are replicated to every core.

Math trick: S = Q K^T = X Wq (X Wk)^T = X M X^T with M = Wq Wk^T computed once
(1024^3 MACs) instead of K = X Wk (2048*1024^2 MACs) — saves ~12% of PE work.

Per-core pipeline (all matmuls fp16 in / fp32 PSUM accumulate, 512-col moving
operands — the PSUM-bank ISA limit):
  prologue:
    Wq^T, Wk^T   via PE transpose of casted loads          [n, d]
    M    = Wq Wk^T        (lhsT=Wq^T, rhs=Wk^T)            [d, e]
    X^T  via PE transpose                                  [d, t]
    A^T  = M^T X^T        (lhsT=M,    rhs=X^T)             [e, t]   (A = X M)
    V    = X Wv           (lhsT=X^T,  rhs=Wv)              [t, n]
  per q-tile (128 rows), software-pipelined by one stage:
    S    = A^T.T X^T      -> PSUM [128, 2048]
    softmax: block-max (DVE) -> exp+row-sum fused on ACT -> P fp16
    P^T  via PE transpose (4 groups of 4, batched ACT copies), transposes
         woven into the S(q+1)/O(q) matmul streams so the PE never waits
    O    = P^T.T V        -> PSUM [128, 1024]; O *= 1/rowsum; DMA out
"""

from contextlib import ExitStack

import numpy as np

import concourse.bass as bass
import concourse.mybir as mybir
import concourse.tile as tile
from concourse import bacc
from concourse.bass import ts
from concourse.masks import make_identity

P = 128
T = 2048
D = 1024
NO = 1024
B = 8
TT = T // P   # 16 tiles of 128 along t
DT = D // P   # 8 tiles along d (and along e)
NT = NO // P  # 8 tiles along nodes

F16 = mybir.dt.float16
F32 = mybir.dt.float32
AX = mybir.AxisListType
EXP = mybir.ActivationFunctionType.Exp


def _attention_body(tc, out, x, wq, wk, wv):
    nc = tc.nc
    x3 = x.rearrange("(t p) d -> t p d", p=P)
    o3 = out.rearrange("(t p) n -> t p n", p=P)

    with ExitStack() as ctx:
        const = ctx.enter_context(tc.tile_pool(name="const", bufs=1))
        persist = ctx.enter_context(tc.tile_pool(name="persist", bufs=1))
        # 1-bank psum pool: V accumulators and P^T transpose staging
        prps = ctx.enter_context(tc.tile_pool(name="prps", bufs=2, space="PSUM"))

        ident = const.tile([P, P], F16, tag="ident")
        make_identity(nc, ident)

        xt = persist.tile([P, DT, T], F16, tag="xt")     # X^T [d, t]
        at = persist.tile([P, DT, T], F16, tag="at")     # A^T [e, t]
        v = persist.tile([P, TT, NO], F16, tag="v")      # V   [t, n]
        m16 = persist.tile([P, DT, NO], F16, tag="m16")  # M   [d, e]
        wv16 = persist.tile([P, DT, NO], F16, tag="wv16")

        # ---------------- prologue ----------------
        with tc.tile_pool(name="wt", bufs=1) as wtp, tc.tile_pool(
            name="stage", bufs=6
        ) as stage, tc.tile_pool(name="tpsum", bufs=2, space="PSUM") as tpsum, tc.tile_pool(
            name="accps", bufs=2, space="PSUM"
        ) as accps:
            wqT = wtp.tile([P, NT, D], F16, tag="wqT")   # Wq^T [n, d]
            wkT = wtp.tile([P, NT, D], F16, tag="wkT")

            # Wq, Wk first (M needs only these): load f32 in 512-col chunks
            # (faster first-byte), cast (GPSIMD), PE-transpose in 4-block
            # groups, batched copy per group alternating DVE/ACT
            cp_flip = [0]

            def group_copy(dst, src):
                if cp_flip[0] % 2 == 0:
                    nc.vector.tensor_copy(dst, src)
                else:
                    nc.scalar.copy(dst, src)
                cp_flip[0] += 1

            cast_flip = [0]

            def cast_copy(dst, src):
                # alternate GPSIMD/DVE so the cast stage outpaces the DMA
                if cast_flip[0] % 2 == 0:
                    nc.gpsimd.tensor_copy(dst, src)
                else:
                    nc.vector.tensor_copy(dst, src)
                cast_flip[0] += 1

            wq3 = wq.rearrange("(do p) n -> do p n", p=P)
            wk3 = wk.rearrange("(do p) n -> do p n", p=P)

            def w_chunk(wap3, dstT, do, h):
                ws = stage.tile([P, 512], F32, tag="ws")
                nc.sync.dma_start(ws, wap3[do][:, ts(h, 512)])
                wh = stage.tile([P, 512], F16, tag="wh")
                cast_copy(wh, ws)
                tp = tpsum.tile([P, 4, P], F16, tag="tp")
                for no in range(4):
                    nc.tensor.transpose(tp[:, no], wh[:, ts(no, P)], ident)
                group_copy(dstT[:, ts(h, 4), ts(do, P)], tp)

            # h=0 halves of BOTH weights first: n-tiles 0-3 of Wq^T/Wk^T are
            # complete after 4MB instead of 8MB, so M pass-0 starts early
            for wap3, dstT in ((wq3, wqT), (wk3, wkT)):
                for do in range(DT):
                    w_chunk(wap3, dstT, do, 0)

            def m_pass(dt_, eb, nts, first, last):
                ps = accps.tile([P, 512], F32, tag="ps")
                for nt in nts:
                    nc.tensor.matmul(
                        ps,
                        wqT[:, nt, ts(dt_, P)],
                        wkT[:, nt, ts(eb, 512)],
                        start=(nt == nts[0]),
                        stop=(nt == nts[-1]),
                    )
                dst = m16[:, dt_, ts(eb, 512)]
                if first:
                    nc.vector.tensor_copy(dst, ps)
                else:
                    nc.vector.tensor_tensor(
                        dst, ps, dst, op=mybir.AluOpType.add
                    )

            # M pass-0 (n-tiles 0-3) interleaved with the h=1 W chunks
            mchunks = [(d_, e_) for d_ in range(DT) for e_ in range(2)]
            for mi, (dt_, eb) in enumerate(mchunks):
                if mi < DT:
                    w_chunk(wq3, wqT, mi, 1)
                    w_chunk(wk3, wkT, mi, 1)
                m_pass(dt_, eb, [0, 1, 2, 3], True, False)

            # M pass-1 (n-tiles 4-7, added into m16) interleaved with X^T
            # groups so the transposes hide inside the matmul stream
            for mi, (dt_, eb) in enumerate(mchunks):
                m_pass(dt_, eb, [4, 5, 6, 7], False, True)
                for j in (2 * mi, 2 * mi + 1):
                    t_, h = divmod(j, 2)
                    xs = stage.tile([P, 512], F32, tag="ws")
                    nc.sync.dma_start(xs, x3[t_][:, ts(h, 512)])
                    xh = stage.tile([P, 512], F16, tag="wh")
                    cast_copy(xh, xs)
                    tp = tpsum.tile([P, 4, P], F16, tag="tp")
                    for do in range(4):
                        nc.tensor.transpose(tp[:, do], xh[:, ts(do, P)], ident)
                    group_copy(xt[:, ts(h, 4), ts(t_, P)], tp)

            # A^T[e, q] = sum_d M[d, e] X^T[d, q]
            for et in range(DT):
                for qb in range(4):
                    ps = accps.tile([P, 512], F32, tag="ps")
                    for dt_ in range(DT):
                        nc.tensor.matmul(
                            ps,
                            m16[:, dt_, ts(et, P)],
                            xt[:, dt_, ts(qb, 512)],
                            start=(dt_ == 0),
                            stop=(dt_ == DT - 1),
                        )
                    nc.vector.tensor_copy(at[:, et, ts(qb, 512)], ps)

            # Wv last (V is the last prologue consumer)
            wv3 = wv.rearrange("(do p) n -> do p n", p=P)
            for do in range(DT):
                ws = stage.tile([P, NO], F32, tag="ws2")
                nc.sync.dma_start(ws, wv3[do])
                nc.gpsimd.tensor_copy(wv16[:, do, :], ws)

        def v_chunk(t_):
            for nb in range(2):
                ps = prps.tile([P, 512], F32, tag="ps")
                for dt_ in range(DT):
                    nc.tensor.matmul(
                        ps,
                        xt[:, dt_, ts(t_, P)],
                        wv16[:, dt_, ts(nb, 512)],
                        start=(dt_ == 0),
                        stop=(dt_ == DT - 1),
                    )
                nc.vector.tensor_copy(v[:, t_, ts(nb, 512)], ps)

        # all but the last two V chunks; those fill the first softmax gap
        for t_ in range(TT - 2):
            v_chunk(t_)

        # ---------------- attention ----------------
        with tc.tile_pool(name="spsum", bufs=1, space="PSUM") as spsum, tc.tile_pool(
            name="opsum", bufs=1, space="PSUM"
        ) as opsum, tc.tile_pool(name="soft", bufs=2) as soft, tc.tile_pool(
            name="ptp", bufs=2
        ) as ptp, tc.tile_pool(name="outp", bufs=2) as outp:

            def pt_group(p16_, ptt, g):
                # P^T group g (4 blocks of 128): PE transpose + one ACT copy
                pt_ps = prps.tile([P, 4, P], F16, tag="ps")
                for j in range(4):
                    nc.tensor.transpose(pt_ps[:, j], p16_[:, g, ts(j, P)], ident)
                # ACT, not DVE: the DVE is busy with the softmax reduces and
                # the O matmuls block on this copy
                nc.scalar.copy(ptt[:, g], pt_ps)

            def emit_o(ptt, o_ps, rsum_, q_, first_k=0):
                for k_ in range(first_k, TT):
                    for nb in range(2):
                        nc.tensor.matmul(
                            o_ps[:, nb],
                            ptt[:, k_ // 4, ts(k_ % 4, P)],
                            v[:, k_, ts(nb, 512)],
                            start=(k_ == 0),
                            stop=(k_ == TT - 1),
                        )
                inv = soft.tile([P, 1], F32, tag="inv")
                nc.vector.reciprocal(inv, rsum_)
                ob = outp.tile([P, NO], F32, tag="ob")
                o2 = o3[q_]
                for nb in range(2):
                    nc.vector.tensor_scalar_mul(ob[:, ts(nb, 512)], o_ps[:, nb], inv)
                    nc.sync.dma_start(o2[:, ts(nb, 512)], ob[:, ts(nb, 512)])

            prev = None
            for q_ in range(TT):
                # PT(q-1) transposes woven into the S(q) matmul stream so the
                # staging copies hide inside the matmul windows; the last
                # group is woven into the O stream.
                if prev is not None:
                    ptt = ptp.tile([P, 4, 4 * P], F16, tag="ptt")
                s = spsum.tile([P, 4, 512], F32, tag="s")
                for et in range(DT):
                    for kb in range(4):
                        nc.tensor.matmul(
                            s[:, kb],
                            at[:, et, ts(q_, P)],
                            xt[:, et, ts(kb, 512)],
                            start=(et == 0),
                            stop=(et == DT - 1),
                        )
                    if prev is not None and et >= 5:
                        pt_group(prev[0], ptt, et - 5)
                if q_ == 0:
                    v_chunk(TT - 2)
                    v_chunk(TT - 1)
                if prev is not None:
                    o_ps = opsum.tile([P, 2, 512], F32, tag="o")
                    for k_ in range(4):
                        for nb in range(2):
                            nc.tensor.matmul(
                                o_ps[:, nb],
                                ptt[:, k_ // 4, ts(k_ % 4, P)],
                                v[:, k_, ts(nb, 512)],
                                start=(k_ == 0),
                                stop=False,
                            )
                    pt_group(prev[0], ptt, 3)
                    emit_o(ptt, o_ps, prev[1], prev[2], first_k=4)
                # fused softmax: one negated max-reduce over the whole S tile,
                # one exp over [P, 4, 512] with the row-sum as accum_out
                negmax = soft.tile([P, 1], F32, tag="negmax")
                nc.vector.tensor_reduce(
                    negmax, s, axis=AX.XY, op=mybir.AluOpType.max, negate=True
                )
                p16 = soft.tile([P, 4, 512], F16, tag="p16")
                rsum = soft.tile([P, 1], F32, tag="rsum")
                nc.scalar.activation(
                    p16, s, EXP, bias=negmax, scale=1.0, accum_out=rsum
                )
                prev = (p16, rsum, q_)
            # tail: last tile's PT + O
            ptt = ptp.tile([P, 4, 4 * P], F16, tag="ptt")
            for g in range(4):
                pt_group(prev[0], ptt, g)
            o_ps = opsum.tile([P, 2, 512], F32, tag="o")
            emit_o(ptt, o_ps, prev[1], prev[2], first_k=0)


_CACHED_NC = {}


def _build(iters=1):
    global _CACHED_NC
    if iters in _CACHED_NC:
        return _CACHED_NC[iters]
    nc = bacc.Bacc("TRN2", target_bir_lowering=False, debug=False, num_devices=1)
    x = nc.dram_tensor("x", (T, D), F32, kind="ExternalInput").ap()
    wq = nc.dram_tensor("wq", (D, NO), F32, kind="ExternalInput").ap()
    wk = nc.dram_tensor("wk", (D, NO), F32, kind="ExternalInput").ap()
    wv = nc.dram_tensor("wv", (D, NO), F32, kind="ExternalInput").ap()
    out = nc.dram_tensor("out", (T, NO), F32, kind="ExternalOutput").ap()
    with tile.TileContext(nc) as tc:
        for _ in range(iters):
            _attention_body(tc, out, x, wq, wk, wv)
    nc.compile()
    _CACHED_NC[iters] = nc
    return nc


def kernel(inputs, Wq, Wk, Wv, trace=False):
    from concourse.bass_utils import run_bass_kernel_spmd

    nc = _build()
    inputs = np.ascontiguousarray(inputs, dtype=np.float32)
    Wq = np.ascontiguousarray(Wq, dtype=np.float32)
    Wk = np.ascontiguousarray(Wk, dtype=np.float32)
    Wv = np.ascontiguousarray(Wv, dtype=np.float32)
    in_maps = [
        {"x": inputs[b], "wq": Wq, "wk": Wk, "wv": Wv} for b in range(B)
    ]
    res = run_bass_kernel_spmd(nc, in_maps, core_ids=list(range(B)), trace=False)
    out = np.stack([r["out"] for r in res.results], axis=0)
    return out
